# revision 1
# baseline (speedup 1.0000x reference)
"""CenterWeightedCIoULoss on 8 Trainium2 NeuronCores (Bass/Tile), v2.

Math per matched pair (xyxy):
    loss = (1 - iou) + 2*center + size,  output = mean(loss)

Host re-encodes each box tensor losslessly-in-f32 then casts fp16:
    A = p1 + p2 (2*center), W = p2 - p1 (width/height), planar layout
    [128, (Ax Ay Wx Wy), nb] per core shard.

On-chip identities (dc = Ap - At, dw = Wp - Wt):
    dc == 2*(pc - tc)           (center delta, feeds center term)
    dw == pw - tw               (feeds size term)
    |d1| + |d2| == max(|dc|, |dw|) = u   (d1 = p1-t1, d2 = p2-t2)
    2*inter_wh = relu(a - u), 2*c_wh = a + u,  a = Wp + Wt
    inter = (s/2)_x * (s/2)_y,  union = ap + at - inter
    center = (dcx^2 + dcy^2) / ((a+u)_x^2 + (a+u)_y^2)   [cd2=dc, cwh2=a+u]
    size = (dw_x/Wt_x)^2 + (dw_y/Wt_y)^2

Engines: DVE does the 2-byte 2x/4x arithmetic and the three reciprocals; ACT
(pattern-blind 1 ns/elem) does Abs/Square; Pool (gpsimd) takes the
input-only products/adds (ap, at, ap+at, Wp+Wt) so it never gates the DVE
chain. Each tile's post-reciprocal tail is emitted one iteration late
(software pipelining) so DVE never stalls on Pool/ACT results.
Accumulation: per-tile f32 columns via tensor_scalar/ACT accum_out; host
sums in f64 and adds the constant 1.
"""

import sys

sys.path.insert(0, "/opt/trn_rl_repo")

import numpy as np

import concourse.bass as bass
import concourse.bacc as bacc
import concourse.tile as tile
from concourse import mybir
from concourse.bass_utils import run_bass_kernel_spmd

# All ACT funcs used here (Abs/Relu/Square/Ln/Exp) live in the single
# 'natural_log_exp_and_others' table set, but bacc's greedy per-instruction
# chooser bounces between sets (one 1283ns table load per switch). Restrict
# the candidates to that set (others kept, emptied, preserving indices).
if getattr(bacc.get_activation_tables, "_ciou_pinned", False):
    _orig_get_tables = bacc.get_activation_tables._ciou_orig
else:
    _orig_get_tables = bacc.get_activation_tables


def _pinned_tables(arch):
    tables = _orig_get_tables(arch)
    pinned = "natural_log_exp_and_others"
    assert pinned in tables
    return {
        name: (funcs if name == pinned else set())
        for name, funcs in tables.items()
    }


_pinned_tables._ciou_pinned = True
_pinned_tables._ciou_orig = _orig_get_tables
bacc.get_activation_tables = _pinned_tables

N = 4_194_304
NCORES = 8
NB = N // NCORES            # boxes per core
P = 128
nb = NB // P                # 4096 boxes per partition
# tile schedule: small edge tiles shorten pipeline fill/drain; tuned on
# TimelineSim (uniform 1024 -> 76.1us, this schedule -> 71.1us)
TILES = [352, 768, 1088, 1056, 832]
assert sum(TILES) == nb
T = len(TILES)
IO_BUFS = 3
MID_BUFS = 2

F32 = mybir.dt.float32
F16 = mybir.dt.float16
BF16 = mybir.dt.bfloat16
Alu = mybir.AluOpType
Act = mybir.ActivationFunctionType

_compiled = None


def _build():
    nc = bacc.Bacc("TRN2", target_bir_lowering=False, debug=False)
    pred = nc.dram_tensor("pred", [P, 4 * nb], F16, kind="ExternalInput").ap()
    targ = nc.dram_tensor("targ", [P, 4 * nb], F16, kind="ExternalInput").ap()
    out = nc.dram_tensor("out", [P, 2 * T], F32, kind="ExternalOutput").ap()

    prv = pred.rearrange("p (c n) -> p c n", c=4)
    tgv = targ.rearrange("p (c n) -> p c n", c=4)

    def c2(t):  # [P, 2*BX] tile -> [P, 2, BX] plane view
        return t[:].rearrange("p (c n) -> p c n", c=2)

    with nc.allow_low_precision(reason="fp16/bf16 pipeline, f32 accumulators"):
        with tile.TileContext(nc) as tc:
            with (
                tc.tile_pool(name="io", bufs=IO_BUFS) as io,
                tc.tile_pool(name="mid", bufs=MID_BUFS) as mid,
                tc.tile_pool(name="accp", bufs=1) as accp,
            ):
                accA = accp.tile([P, T], F32)
                accB = accp.tile([P, T], F32)
                pend = None  # deferred tail state from previous tile

                def head(t, n0, bx, rty_act=True, relu_act=True):
                    BX = bx
                    sl = slice(n0, n0 + bx)
                    pa = io.tile([P, 4 * BX], F16, tag="pa")
                    ta = io.tile([P, 4 * BX], F16, tag="ta")
                    nc.sync.dma_start(
                        pa[:].rearrange("p (c n) -> p c n", c=4), prv[:, :, sl]
                    )
                    nc.sync.dma_start(
                        ta[:].rearrange("p (c n) -> p c n", c=4), tgv[:, :, sl]
                    )
                    pav = pa[:].rearrange("p (c n) -> p c n", c=4)
                    tav = ta[:].rearrange("p (c n) -> p c n", c=4)
                    Ap, Wp = pav[:, 0:2], pav[:, 2:4]
                    At, Wt = tav[:, 0:2], tav[:, 2:4]

                    # Pool: products/sums straight off the inputs (never gate DVE)
                    ap_ = mid.tile([P, BX], F16, tag="ap")
                    at_ = mid.tile([P, BX], F16, tag="at")
                    u4 = mid.tile([P, BX], F16, tag="u4")
                    a_ = mid.tile([P, 2 * BX], F16, tag="a")
                    nc.gpsimd.tensor_tensor(c2(a_), Wp, Wt, Alu.add)
                    nc.gpsimd.tensor_tensor(ap_[:], Wp[:, 0], Wp[:, 1], Alu.mult)
                    nc.gpsimd.tensor_tensor(at_[:], Wt[:, 0], Wt[:, 1], Alu.mult)
                    nc.gpsimd.tensor_tensor(u4[:], ap_[:], at_[:], Alu.add)

                    # DVE: one full-width sub gives dc (A halves) and dw (W halves)
                    dd = mid.tile([P, 4 * BX], F16, tag="dd")
                    nc.vector.tensor_sub(dd[:], pa[:], ta[:])
                    dc = dd[:, 0 : 2 * BX]
                    dw = dd[:, 2 * BX : 4 * BX]

                    bb = mid.tile([P, 4 * BX], F16, tag="bb")
                    nc.scalar.activation(bb[:], dd[:], Act.Abs)
                    bc = bb[:, 0 : 2 * BX]
                    bw = bb[:, 2 * BX : 4 * BX]

                    # 1/Wt split across engines: x-plane on DVE reciprocal,
                    # y-plane on ACT as exp(-ln) (same act table as Abs/Square)
                    rtw = mid.tile([P, 2 * BX], F16, tag="rtw")
                    rv = c2(rtw)
                    if rty_act:
                        nc.scalar.activation(rv[:, 1], Wt[:, 1], Act.Ln)
                        nc.scalar.activation(rv[:, 1], rv[:, 1], Act.Exp, scale=-1.0)
                        nc.vector.reciprocal(rv[:, 0], Wt[:, 0])
                    else:
                        nc.vector.reciprocal(rv, Wt)
                    q = mid.tile([P, 2 * BX], F16, tag="q")
                    nc.vector.tensor_mul(q[:], dw, rtw[:])
                    nc.scalar.activation(
                        q[:], q[:], Act.Square,
                        accum_out=accB[:, t : t + 1],
                    )

                    u = mid.tile([P, 2 * BX], F16, tag="u")
                    nc.vector.tensor_tensor(u[:], bc, bw, Alu.max)
                    s = mid.tile([P, 2 * BX], F16, tag="s")
                    cwh2 = mid.tile([P, 2 * BX], F16, tag="cwh2")
                    nc.vector.tensor_sub(s[:], a_[:], u[:])
                    nc.vector.tensor_add(cwh2[:], a_[:], u[:])
                    if relu_act:
                        nc.scalar.activation(s[:], s[:], Act.Relu, scale=0.5)
                    else:
                        nc.vector.tensor_scalar(s[:], s[:], 0.5, 0.0, Alu.mult, Alu.max)
                    sv = c2(s)
                    na = mid.tile([P, 2 * BX], BF16, tag="na")
                    nb = mid.tile([P, 2 * BX], BF16, tag="nb")
                    nav, nbv = c2(na), c2(nb)
                    nc.vector.tensor_mul(nav[:, 0], sv[:, 0], sv[:, 1])  # inter
                    nc.vector.tensor_sub(nbv[:, 0], nav[:, 0], u4[:])  # -union

                    sqcd = mid.tile([P, 2 * BX], BF16, tag="sqcd")
                    sqcw = mid.tile([P, 2 * BX], BF16, tag="sqcw")
                    nc.scalar.activation(sqcd[:], dc, Act.Square, scale=1.4142135)
                    nc.scalar.activation(sqcw[:], cwh2[:], Act.Square)
                    scv, swv = c2(sqcd), c2(sqcw)
                    nc.vector.tensor_add(nav[:, 1], scv[:, 0], scv[:, 1])  # cdsq
                    nc.vector.tensor_add(nbv[:, 1], swv[:, 0], swv[:, 1])  # cdiag
                    return (t, na, nb, bx)

                def tail(st, rby_act=False):
                    t, na, nb, BX = st
                    # rb = 1/(-union | cdiag); ioct = (-iou | 2*center)
                    if rby_act:
                        nbv = c2(nb)
                        nc.scalar.activation(nbv[:, 1], nbv[:, 1], Act.Ln)
                        nc.scalar.activation(nbv[:, 1], nbv[:, 1], Act.Exp, scale=-1.0)
                        nc.vector.reciprocal(nbv[:, 0], nbv[:, 0])
                    else:
                        nc.vector.reciprocal(nb[:], nb[:])
                    nc.vector.tensor_mul(na[:], na[:], nb[:])
                    nc.vector.tensor_scalar(
                        na[:], na[:], 1.0, None, Alu.mult, Alu.add,
                        accum_out=accA[:, t : t + 1],
                    )

                # Per-tile engine choices (TimelineSim-tuned): which tiles
                # compute 1/Wt_y via ACT exp(-ln), relu on ACT vs DVE, and
                # 1/cdiag via ACT exp(-ln) in the tail.
                _rty = "01111"
                _rel = "01000"
                _rby = "00111"
                n0 = 0
                for t in range(T):
                    st = head(t, n0, TILES[t], rty_act=_rty[t] == "1",
                              relu_act=_rel[t] == "1")
                    n0 += TILES[t]
                    if pend is not None:
                        tail(pend, rby_act=_rby[pend[0]] == "1")
                    pend = st
                tail(pend, rby_act=_rby[pend[0]] == "1")
                nc.sync.dma_start(out[:, 0:T], accA[:])
                nc.sync.dma_start(out[:, T : 2 * T], accB[:])
    nc.compile()
    return nc


def _encode(boxes: np.ndarray) -> list[np.ndarray]:
    """Split into 8 shards, re-encode each as [P, 4*nb] fp16 planar (A, W)."""
    b = np.ascontiguousarray(boxes, np.float32)
    shards = []
    for c in range(NCORES):
        sh = b[c * NB : (c + 1) * NB]
        arr = np.empty((P, 4, nb), np.float16)
        p1 = sh[:, 0:2].reshape(P, nb, 2)
        p2 = sh[:, 2:4].reshape(P, nb, 2)
        A = p1 + p2
        W = p2 - p1
        arr[:, 0] = A[:, :, 0]
        arr[:, 1] = A[:, :, 1]
        arr[:, 2] = W[:, :, 0]
        arr[:, 3] = W[:, :, 1]
        shards.append(arr.reshape(P, 4 * nb))
    return shards


def kernel(pred_boxes: np.ndarray, target_boxes: np.ndarray) -> np.ndarray:
    global _compiled
    if _compiled is None:
        _compiled = _build()
    nc = _compiled
    preds = _encode(pred_boxes)
    targs = _encode(target_boxes)
    in_maps = [{"pred": preds[i], "targ": targs[i]} for i in range(NCORES)]
    res = run_bass_kernel_spmd(nc, in_maps, core_ids=list(range(NCORES))).results
    total = 0.0
    for r in res:
        total += np.sum(r["out"].astype(np.float64))
    return np.float32(1.0 + total / N)

